# revision 6
# baseline (speedup 1.0000x reference)
"""Trainium2 Bass kernel for nn_BatchHighOrderActivation.

Reference semantics (per batch b, channel g):
    sort the ARITY=4 values x = X[b,g,:], build barycentric coefficients from
    the sorted gaps, gather params rows by reverse-cumsum bitmasks, contract.

Sort/gather-free reformulation (multilinear simplex / Lovasz form):
    out[b,g,:] = sum_{m=1..15} w[b,g,m] * params[g,m,:]
    w[m] = relu( min_{i in m} x_i - max_{i not in m} x_i )   for m != 15
    w[15] = min_i x_i                                        (no relu)

Kernel structure per core (pure batch data-parallel sharding, 512 rows/core),
fp16 internal compute and fp16 DRAM output (host casts back to fp32; adds
~2e-4 absmax-relative error, total ~7e-4, well under the 2e-3 gate):
  - host: X de-interleaved to fp16 arity-planes; params expanded to an fp16
          block-diagonal table (8 channels/group, K-order (m,gl))
  - GPS : pair mins, pair maxes, triple maxes (16 of 36 tree ops) on GpSimd
          to offload the DVE; W m0-column memset
  - DVE : triple mins, quad min (written straight into the m15 slots), the
          14 subtractions as single FD=512 ops, one relu pass
  - PE  : transpose W 128x128 chunks via fp16 identity matmul (16 chunks per
          PSUM tile), block-diagonal fp16 matmul (K=(m,gl)=128, N=4ch*256),
          fp32 PSUM
  - DVE : W^T evacuation as FD=2048 fp16 copies (2x mode)
  - ACT : out evacuation PSUM->SBUF with fused fp32->fp16 cast
  - DMA : input loads ride SWDGE (gpsimd) queues; fp16 stores on HWDGE (sync)
Baseline (fp32 out, all-DVE tree, per-half subs): 122.5us.
"""

import numpy as np
from contextlib import ExitStack

import concourse.bass as bass
import concourse.mybir as mybir
import concourse.tile as tile
from concourse import bacc
from concourse.bass_utils import run_bass_kernel_spmd
from concourse.masks import make_identity

F32 = mybir.dt.float32
F16 = mybir.dt.float16
NCORES = 8
B, G, A, O = 4096, 512, 4, 32
BS = B // NCORES        # 512 batch rows per core
NBT = BS // 128         # 4 b-tiles per core
NQ = G // 8             # 64 channel groups of 8

_PAIRS = [(0, 1), (0, 2), (0, 3), (1, 2), (1, 3), (2, 3)]
_TRIPLES = [(0, 1, 2), (0, 1, 3), (0, 2, 3), (1, 2, 3)]

_cached_nc = None


def _build_program():
    nc = bacc.Bacc("TRN2", target_bir_lowering=False, debug=False, num_devices=NCORES)

    x_d = nc.dram_tensor("x", [BS, A * G], F16, kind="ExternalInput").ap()
    pbd_d = nc.dram_tensor("pbd", [128, NQ * 256], F16, kind="ExternalInput").ap()
    out_d = nc.dram_tensor("out", [BS, G * O], F16, kind="ExternalOutput").ap()

    with ExitStack() as ctx:
        tc = ctx.enter_context(tile.TileContext(nc))
        persist = ctx.enter_context(tc.tile_pool(name="persist", bufs=1))
        plpool = ctx.enter_context(tc.tile_pool(name="pl", bufs=3))
        treep = ctx.enter_context(tc.tile_pool(name="tree", bufs=2))
        wpool = ctx.enter_context(tc.tile_pool(name="w", bufs=2))
        lhsp = ctx.enter_context(tc.tile_pool(name="lt", bufs=3))
        stgp = ctx.enter_context(tc.tile_pool(name="stg", bufs=3))
        ptp = ctx.enter_context(tc.tile_pool(name="pt", bufs=2, space="PSUM"))
        pmp = ctx.enter_context(tc.tile_pool(name="pm", bufs=2, space="PSUM"))

        pbd = persist.tile([128, NQ * 256], F16)
        pl0 = plpool.tile([128, A, G], F16, tag="pl")
        identity = persist.tile([128, 128], F16)
        nc.gpsimd.dma_start(pl0[:].rearrange("p a g -> p (a g)"), x_d[0:128, :])
        nc.gpsimd.dma_start(pbd[:], pbd_d[:])
        make_identity(nc, identity[:])

        oev = [0]
        for bt in range(NBT):
            if bt == 0:
                pl = pl0
            else:
                pl = plpool.tile([128, A, G], F16, tag="pl")
                nc.gpsimd.dma_start(
                    pl[:].rearrange("p a g -> p (a g)"), x_d[bt * 128:(bt + 1) * 128, :]
                )
            s = [pl[:, i, :] for i in range(A)]

            # tree planes: 6 mn2, 6 mx2, 4 mn3, 4 mx3 (mn4 goes into W's m15)
            tr = treep.tile([128, 20, G], F16, tag="tree")
            slot = [0]
            mn, mx = {}, {}

            def alloc():
                ap = tr[:, slot[0], :]
                slot[0] += 1
                return ap

            # min/max tree on DVE (GpSimd's Q7 ucode lacks min/max)
            for (i, j) in _PAIRS:
                mn[(i, j)] = alloc()
                nc.vector.tensor_tensor(mn[(i, j)], s[i], s[j], mybir.AluOpType.min)
            for (i, j) in _PAIRS:
                mx[(i, j)] = alloc()
                nc.vector.tensor_tensor(mx[(i, j)], s[i], s[j], mybir.AluOpType.max)
            for (i, j, k) in _TRIPLES:
                mn[(i, j, k)] = alloc()
                nc.vector.tensor_tensor(mn[(i, j, k)], mn[(i, j)], s[k], mybir.AluOpType.min)
            for (i, j, k) in _TRIPLES:
                mx[(i, j, k)] = alloc()
                nc.vector.tensor_tensor(mx[(i, j, k)], mx[(i, j)], s[k], mybir.AluOpType.max)

            def sub_ap(S):
                return s[S[0]] if len(S) == 1 else mn[S]

            def sup_ap(Cm):
                return s[Cm[0]] if len(Cm) == 1 else mx[Cm]

            # W layout: free = q*128 + m*8 + gl (K-order (m,gl)); sub writes
            # are 8-element step-1 runs spanning all 64 q (FD=512, 2x mode),
            # transpose inputs are contiguous 128-column blocks.
            wt = wpool.tile([128, NQ * 128], F16, tag="w")
            wv4 = wt.rearrange("p (q m gl) -> p q m gl", m=16, gl=8)
            wvr = wt.rearrange("p (q r) -> p q r", r=128)
            # m=0 columns: matmul contribution is zeroed by the zero params
            # rows but must be finite
            nc.gpsimd.memset(wv4[:, :, 0, :], 0.0)
            # the subtractions are the one tree stage GpSimd's ucode supports;
            # offload most of them (split knob: which m stay on DVE)
            for m in range(1, 15):
                S = tuple(i for i in range(A) if (m >> i) & 1)
                Cm = tuple(i for i in range(A) if not ((m >> i) & 1))
                eng = nc.vector if m in (1, 2, 3) else nc.gpsimd
                eng.tensor_tensor(
                    wv4[:, :, m, :], sub_ap(S), sup_ap(Cm), mybir.AluOpType.subtract
                )
            # m15 = min of all four, written directly (no relu applied to it)
            nc.vector.tensor_tensor(
                wv4[:, :, 15, :], mn[(0, 1, 2)], s[3], mybir.AluOpType.min
            )
            # relu m=1..14 in one pass (112-wide step-1 runs, 4x mode)
            nc.vector.tensor_scalar_max(wvr[:, :, 8:120], wvr[:, :, 8:120], 0.0)

            for qg in range(4):          # 4 groups of 16 channel-groups
                pt = ptp.tile([128, 16 * 128], F16, tag="pt")
                for j in range(16):
                    q = qg * 16 + j
                    nc.tensor.transpose(
                        pt[:, j * 128:(j + 1) * 128],
                        wt[:, q * 128:(q + 1) * 128],
                        identity[:],
                    )
                lt = lhsp.tile([128, 16 * 128], F16, tag="lt")
                nc.vector.tensor_copy(lt[:], pt[:])

                stg = stgp.tile([128, 16 * 256], F16, tag="stg")
                for seg in range(4):
                    pm = pmp.tile([128, 1024], F32, tag="pm")
                    for j2 in range(4):
                        j = seg * 4 + j2
                        q = qg * 16 + j
                        nc.tensor.matmul(
                            pm[:, j2 * 256:(j2 + 1) * 256],
                            lt[:, j * 128:(j + 1) * 128],
                            pbd[:, q * 256:(q + 1) * 256],
                            start=True,
                            stop=True,
                        )
                    dst = stg[:, seg * 1024:(seg + 1) * 1024]
                    if oev[0] % 16 == 13:
                        nc.vector.tensor_copy(dst, pm[:])
                    else:
                        nc.scalar.copy(dst, pm[:])
                    oev[0] += 1
                q0 = qg * 16
                nc.sync.dma_start(
                    out_d[bt * 128:(bt + 1) * 128, q0 * 256:(q0 + 16) * 256],
                    stg[:],
                )

    nc.compile()
    return nc


def _get_program():
    global _cached_nc
    if _cached_nc is None:
        _cached_nc = _build_program()
    return _cached_nc


def _make_inputs(X, params):
    X = np.ascontiguousarray(X, dtype=np.float32)
    params = np.ascontiguousarray(params, dtype=np.float32)
    P4 = params.reshape(NQ, 8, 16, O)                 # [q, gl, m, o]
    # block-diag table: pbd[m*8+gl, q*256 + gl*32 + o] = params[8q+gl, m, o]
    Pb = np.zeros((16, 8, NQ, 8, O), np.float32)
    for gl in range(8):
        Pb[1:, gl, :, gl, :] = P4[:, gl, 1:, :].transpose(1, 0, 2)
    pbd = np.ascontiguousarray(Pb.reshape(128, NQ * 256).astype(np.float16))
    # de-interleave X to per-arity fp16 planes: [B, G, A] -> [B, A, G]
    Xp = np.ascontiguousarray(
        X.reshape(B, G, A).transpose(0, 2, 1).astype(np.float16).reshape(B, A * G)
    )
    in_maps = [
        {"x": Xp[c * BS:(c + 1) * BS], "pbd": pbd}
        for c in range(NCORES)
    ]
    return in_maps


def kernel(X, params):
    nc = _get_program()
    in_maps = _make_inputs(X, params)
    res = run_bass_kernel_spmd(nc, in_maps, list(range(NCORES))).results
    out = np.concatenate(
        [res[c]["out"].astype(np.float32).reshape(BS, G, O) for c in range(NCORES)],
        axis=0,
    )
    return out


def kernel_traced(X, params):
    """Like kernel() but also returns the BassKernelResults (profile info)."""
    nc = _get_program()
    in_maps = _make_inputs(X, params)
    br = run_bass_kernel_spmd(nc, in_maps, list(range(NCORES)), trace=True)
    out = np.concatenate(
        [br.results[c]["out"].astype(np.float32).reshape(BS, G, O)
         for c in range(NCORES)],
        axis=0,
    )
    return out, br


# revision 9
# speedup vs baseline: 1.1896x; 1.1896x over previous
"""Trainium2 Bass kernel for nn_BatchHighOrderActivation.

Reference semantics (per batch b, channel g):
    sort the ARITY=4 values x = X[b,g,:], build barycentric coefficients from
    the sorted gaps, gather params rows by reverse-cumsum bitmasks, contract.

Sort/gather-free reformulation (multilinear simplex / Lovasz form):
    out[b,g,:] = sum_{m=1..15} w[b,g,m] * params[g,m,:]
    w[m] = relu( min_{i in m} x_i - max_{i not in m} x_i )   for m != 15
    w[15] = min_i x_i                                        (no relu)

Kernel structure per core (pure batch data-parallel sharding, 512 rows/core),
fp16 internal compute and fp16 DRAM output (host casts back to fp32; adds
~2e-4 absmax-relative error, total ~7e-4, well under the 2e-3 gate):
  - host: X de-interleaved to fp16 arity-planes; params expanded to an fp16
          block-diagonal table (8 channels/group, K-order (m,gl))
  - GPS : pair mins, pair maxes, triple maxes (16 of 36 tree ops) on GpSimd
          to offload the DVE; W m0-column memset
  - DVE : triple mins, quad min (written straight into the m15 slots), the
          14 subtractions as single FD=512 ops, one relu pass
  - PE  : transpose W 128x128 chunks via fp16 identity matmul (16 chunks per
          PSUM tile), block-diagonal fp16 matmul (K=(m,gl)=128, N=4ch*256),
          fp32 PSUM
  - DVE : W^T evacuation as FD=2048 fp16 copies (2x mode)
  - ACT : out evacuation PSUM->SBUF with fused fp32->fp16 cast
  - DMA : input loads ride SWDGE (gpsimd) queues; fp16 stores on HWDGE (sync)
Baseline (fp32 out, all-DVE tree, per-half subs): 122.5us.
"""

import numpy as np
from contextlib import ExitStack

import concourse.bass as bass
import concourse.mybir as mybir
import concourse.tile as tile
from concourse import bacc
from concourse.bass_utils import run_bass_kernel_spmd
from concourse.masks import make_identity

F32 = mybir.dt.float32
F16 = mybir.dt.float16
NCORES = 8
B, G, A, O = 4096, 512, 4, 32
BS = B // NCORES        # 512 batch rows per core
NBT = BS // 128         # 4 b-tiles per core
NQ = G // 8             # 64 channel groups of 8

_PAIRS = [(0, 1), (0, 2), (0, 3), (1, 2), (1, 3), (2, 3)]
_TRIPLES = [(0, 1, 2), (0, 1, 3), (0, 2, 3), (1, 2, 3)]

_cached_nc = None


def _build_program():
    nc = bacc.Bacc("TRN2", target_bir_lowering=False, debug=False, num_devices=NCORES)

    x_d = nc.dram_tensor("x", [BS, A * G], F16, kind="ExternalInput").ap()
    pbd_d = nc.dram_tensor("pbd", [128, NQ * 256], F16, kind="ExternalInput").ap()
    out_d = nc.dram_tensor("out", [BS, G * O], F16, kind="ExternalOutput").ap()

    with ExitStack() as ctx:
        tc = ctx.enter_context(tile.TileContext(nc))
        persist = ctx.enter_context(tc.tile_pool(name="persist", bufs=1))
        plpool = ctx.enter_context(tc.tile_pool(name="pl", bufs=3))
        treep = ctx.enter_context(tc.tile_pool(name="tree", bufs=2))
        wpool = ctx.enter_context(tc.tile_pool(name="w", bufs=2))
        lhsp = ctx.enter_context(tc.tile_pool(name="lt", bufs=3))
        stgp = ctx.enter_context(tc.tile_pool(name="stg", bufs=3))
        ptp = ctx.enter_context(tc.tile_pool(name="pt", bufs=2, space="PSUM"))
        pmp = ctx.enter_context(tc.tile_pool(name="pm", bufs=2, space="PSUM"))

        pbd = persist.tile([128, NQ * 256], F16)
        pl0 = plpool.tile([128, A, G], F16, tag="pl")
        identity = persist.tile([128, 128], F16)
        nc.gpsimd.dma_start(pl0[:].rearrange("p a g -> p (a g)"), x_d[0:128, :])
        nc.gpsimd.dma_start(pbd[:], pbd_d[:])
        make_identity(nc, identity[:])

        oev = [0]
        for bt in range(NBT):
            if bt == 0:
                pl = pl0
            else:
                pl = plpool.tile([128, A, G], F16, tag="pl")
                nc.gpsimd.dma_start(
                    pl[:].rearrange("p a g -> p (a g)"), x_d[bt * 128:(bt + 1) * 128, :]
                )
            s = [pl[:, i, :] for i in range(A)]

            # tree planes: 6 mn2, 6 mx2, 4 mn3, 4 mx3 (mn4 goes into W's m15)
            tr = treep.tile([128, 20, G], F16, tag="tree")
            slot = [0]
            mn, mx = {}, {}

            def alloc():
                ap = tr[:, slot[0], :]
                slot[0] += 1
                return ap

            # min/max tree on DVE (GpSimd's Q7 ucode lacks min/max)
            for (i, j) in _PAIRS:
                mn[(i, j)] = alloc()
                nc.vector.tensor_tensor(mn[(i, j)], s[i], s[j], mybir.AluOpType.min)
            for (i, j) in _PAIRS:
                mx[(i, j)] = alloc()
                nc.vector.tensor_tensor(mx[(i, j)], s[i], s[j], mybir.AluOpType.max)
            for (i, j, k) in _TRIPLES:
                mn[(i, j, k)] = alloc()
                nc.vector.tensor_tensor(mn[(i, j, k)], mn[(i, j)], s[k], mybir.AluOpType.min)
            for (i, j, k) in _TRIPLES:
                mx[(i, j, k)] = alloc()
                nc.vector.tensor_tensor(mx[(i, j, k)], mx[(i, j)], s[k], mybir.AluOpType.max)

            def sub_ap(S):
                return s[S[0]] if len(S) == 1 else mn[S]

            def sup_ap(Cm):
                return s[Cm[0]] if len(Cm) == 1 else mx[Cm]

            # W layout: free = q*128 + m*8 + gl (K-order (m,gl)); sub writes
            # are 8-element step-1 runs spanning all 64 q (FD=512, 2x mode),
            # transpose inputs are contiguous 128-column blocks.
            wt = wpool.tile([128, NQ * 128], F16, tag="w")
            wv4 = wt.rearrange("p (q m gl) -> p q m gl", m=16, gl=8)
            wvr = wt.rearrange("p (q r) -> p q r", r=128)
            # m=0 columns: matmul contribution is zeroed by the zero params
            # rows but must be finite
            nc.gpsimd.memset(wv4[:, :, 0, :], 0.0)
            # NOTE: concurrent GpSimd tensor ops contend for SBUF ports and
            # drop DVE throughput ~3x (measured); keep all tensor work on DVE
            for m in range(1, 15):
                S = tuple(i for i in range(A) if (m >> i) & 1)
                Cm = tuple(i for i in range(A) if not ((m >> i) & 1))
                nc.vector.tensor_tensor(
                    wv4[:, :, m, :], sub_ap(S), sup_ap(Cm), mybir.AluOpType.subtract
                )
            # m15 = min of all four, written directly (no relu applied to it)
            nc.vector.tensor_tensor(
                wv4[:, :, 15, :], mn[(0, 1, 2)], s[3], mybir.AluOpType.min
            )
            # relu m=1..14 per q-half ([32,112] measured faster than [64,112])
            nc.vector.tensor_scalar_max(wvr[:, 0:32, 8:120], wvr[:, 0:32, 8:120], 0.0)
            nc.vector.tensor_scalar_max(wvr[:, 32:64, 8:120], wvr[:, 32:64, 8:120], 0.0)

            for qg in range(4):          # 4 groups of 16 channel-groups
                pt = ptp.tile([128, 16 * 128], F16, tag="pt")
                for j in range(16):
                    q = qg * 16 + j
                    nc.tensor.transpose(
                        pt[:, j * 128:(j + 1) * 128],
                        wt[:, q * 128:(q + 1) * 128],
                        identity[:],
                    )
                lt = lhsp.tile([128, 16 * 128], F16, tag="lt")
                # W^T evac split: half DVE, half ACT
                if qg % 2 == 0:
                    nc.vector.tensor_copy(lt[:], pt[:])
                else:
                    nc.scalar.copy(lt[:], pt[:])

                stg = stgp.tile([128, 16 * 256], F16, tag="stg")
                for seg in range(4):
                    pm = pmp.tile([128, 1024], F32, tag="pm")
                    for j2 in range(4):
                        j = seg * 4 + j2
                        q = qg * 16 + j
                        nc.tensor.matmul(
                            pm[:, j2 * 256:(j2 + 1) * 256],
                            lt[:, j * 128:(j + 1) * 128],
                            pbd[:, q * 256:(q + 1) * 256],
                            start=True,
                            stop=True,
                        )
                    dst = stg[:, seg * 1024:(seg + 1) * 1024]
                    if oev[0] % 16 == 13:
                        nc.vector.tensor_copy(dst, pm[:])
                    else:
                        nc.scalar.copy(dst, pm[:])
                    oev[0] += 1
                q0 = qg * 16
                nc.sync.dma_start(
                    out_d[bt * 128:(bt + 1) * 128, q0 * 256:(q0 + 16) * 256],
                    stg[:],
                )

    nc.compile()
    return nc


def _get_program():
    global _cached_nc
    if _cached_nc is None:
        _cached_nc = _build_program()
    return _cached_nc


def _make_inputs(X, params):
    X = np.ascontiguousarray(X, dtype=np.float32)
    params = np.ascontiguousarray(params, dtype=np.float32)
    P4 = params.reshape(NQ, 8, 16, O)                 # [q, gl, m, o]
    # block-diag table: pbd[m*8+gl, q*256 + gl*32 + o] = params[8q+gl, m, o]
    Pb = np.zeros((16, 8, NQ, 8, O), np.float32)
    for gl in range(8):
        Pb[1:, gl, :, gl, :] = P4[:, gl, 1:, :].transpose(1, 0, 2)
    pbd = np.ascontiguousarray(Pb.reshape(128, NQ * 256).astype(np.float16))
    # de-interleave X to per-arity fp16 planes: [B, G, A] -> [B, A, G]
    Xp = np.ascontiguousarray(
        X.reshape(B, G, A).transpose(0, 2, 1).astype(np.float16).reshape(B, A * G)
    )
    in_maps = [
        {"x": Xp[c * BS:(c + 1) * BS], "pbd": pbd}
        for c in range(NCORES)
    ]
    return in_maps


def kernel(X, params):
    nc = _get_program()
    in_maps = _make_inputs(X, params)
    res = run_bass_kernel_spmd(nc, in_maps, list(range(NCORES))).results
    out = np.concatenate(
        [res[c]["out"].astype(np.float32).reshape(BS, G, O) for c in range(NCORES)],
        axis=0,
    )
    return out


def kernel_traced(X, params):
    """Like kernel() but also returns the BassKernelResults (profile info)."""
    nc = _get_program()
    in_maps = _make_inputs(X, params)
    br = run_bass_kernel_spmd(nc, in_maps, list(range(NCORES)), trace=True)
    out = np.concatenate(
        [br.results[c]["out"].astype(np.float32).reshape(BS, G, O)
         for c in range(NCORES)],
        axis=0,
    )
    return out, br


# revision 12
# speedup vs baseline: 1.1912x; 1.0014x over previous
"""Trainium2 Bass kernel for nn_BatchHighOrderActivation.

Reference semantics (per batch b, channel g):
    sort the ARITY=4 values x = X[b,g,:], build barycentric coefficients from
    the sorted gaps, gather params rows by reverse-cumsum bitmasks, contract.

Sort/gather-free reformulation (multilinear simplex / Lovasz form):
    out[b,g,:] = sum_{m=1..15} w[b,g,m] * params[g,m,:]
    w[m] = relu( min_{i in m} x_i - max_{i not in m} x_i )   for m != 15
    w[15] = min_i x_i                                        (no relu)

Kernel structure per core (pure batch data-parallel sharding, 512 rows/core),
fp16 internal compute and fp16 DRAM output (host casts back to fp32; adds
~2e-4 absmax-relative error, total ~7e-4, well under the 2e-3 gate):
  - host: X de-interleaved to fp16 arity-planes; params expanded to an fp16
          block-diagonal table (8 channels/group, K-order (m,gl))
  - GPS : pair mins, pair maxes, triple maxes (16 of 36 tree ops) on GpSimd
          to offload the DVE; W m0-column memset
  - DVE : triple mins, quad min (written straight into the m15 slots), the
          14 subtractions as single FD=512 ops, one relu pass
  - PE  : transpose W 128x128 chunks via fp16 identity matmul (16 chunks per
          PSUM tile), block-diagonal fp16 matmul (K=(m,gl)=128, N=4ch*256),
          fp32 PSUM
  - DVE : W^T evacuation as FD=2048 fp16 copies (2x mode)
  - ACT : out evacuation PSUM->SBUF with fused fp32->fp16 cast
  - DMA : input loads ride SWDGE (gpsimd) queues; fp16 stores on HWDGE (sync)
Baseline (fp32 out, all-DVE tree, per-half subs): 122.5us.
"""

import numpy as np
from contextlib import ExitStack

import concourse.bass as bass
import concourse.mybir as mybir
import concourse.tile as tile
from concourse import bacc
from concourse.bass_utils import run_bass_kernel_spmd
from concourse.masks import make_identity

F32 = mybir.dt.float32
F16 = mybir.dt.float16
NCORES = 8
B, G, A, O = 4096, 512, 4, 32
BS = B // NCORES        # 512 batch rows per core
NBT = BS // 128         # 4 b-tiles per core
NQ = G // 8             # 64 channel groups of 8

_PAIRS = [(0, 1), (0, 2), (0, 3), (1, 2), (1, 3), (2, 3)]
_TRIPLES = [(0, 1, 2), (0, 1, 3), (0, 2, 3), (1, 2, 3)]

_cached_nc = None


def _build_program():
    nc = bacc.Bacc("TRN2", target_bir_lowering=False, debug=False, num_devices=NCORES)

    x_d = nc.dram_tensor("x", [BS, A * G], F16, kind="ExternalInput").ap()
    pbd_d = nc.dram_tensor("pbd", [128, NQ * 256], F16, kind="ExternalInput").ap()
    out_d = nc.dram_tensor("out", [BS, G * O], F16, kind="ExternalOutput").ap()

    with ExitStack() as ctx:
        tc = ctx.enter_context(tile.TileContext(nc))
        persist = ctx.enter_context(tc.tile_pool(name="persist", bufs=1))
        plpool = ctx.enter_context(tc.tile_pool(name="pl", bufs=3))
        treep = ctx.enter_context(tc.tile_pool(name="tree", bufs=2))
        wpool = ctx.enter_context(tc.tile_pool(name="w", bufs=2))
        lhsp = ctx.enter_context(tc.tile_pool(name="lt", bufs=3))
        stgp = ctx.enter_context(tc.tile_pool(name="stg", bufs=3))
        ptp = ctx.enter_context(tc.tile_pool(name="pt", bufs=1, space="PSUM"))
        pmp = ctx.enter_context(tc.tile_pool(name="pm", bufs=3, space="PSUM"))

        pbd = persist.tile([128, NQ * 256], F16)
        pl0 = plpool.tile([128, A, G], F16, tag="pl")
        identity = persist.tile([128, 128], F16)
        nc.gpsimd.dma_start(pl0[:].rearrange("p a g -> p (a g)"), x_d[0:128, :])
        nc.gpsimd.dma_start(pbd[:], pbd_d[:])
        make_identity(nc, identity[:])

        oev = [0]
        for bt in range(NBT):
            if bt == 0:
                pl = pl0
            else:
                pl = plpool.tile([128, A, G], F16, tag="pl")
                nc.gpsimd.dma_start(
                    pl[:].rearrange("p a g -> p (a g)"), x_d[bt * 128:(bt + 1) * 128, :]
                )
            s = [pl[:, i, :] for i in range(A)]

            # tree planes: 6 mn2, 6 mx2, 4 mn3, 4 mx3 (mn4 goes into W's m15)
            tr = treep.tile([128, 20, G], F16, tag="tree")
            slot = [0]
            mn, mx = {}, {}

            def alloc():
                ap = tr[:, slot[0], :]
                slot[0] += 1
                return ap

            # min/max tree on DVE (GpSimd's Q7 ucode lacks min/max)
            for (i, j) in _PAIRS:
                mn[(i, j)] = alloc()
                nc.vector.tensor_tensor(mn[(i, j)], s[i], s[j], mybir.AluOpType.min)
            for (i, j) in _PAIRS:
                mx[(i, j)] = alloc()
                nc.vector.tensor_tensor(mx[(i, j)], s[i], s[j], mybir.AluOpType.max)
            for (i, j, k) in _TRIPLES:
                mn[(i, j, k)] = alloc()
                nc.vector.tensor_tensor(mn[(i, j, k)], mn[(i, j)], s[k], mybir.AluOpType.min)
            for (i, j, k) in _TRIPLES:
                mx[(i, j, k)] = alloc()
                nc.vector.tensor_tensor(mx[(i, j, k)], mx[(i, j)], s[k], mybir.AluOpType.max)

            def sub_ap(S):
                return s[S[0]] if len(S) == 1 else mn[S]

            def sup_ap(Cm):
                return s[Cm[0]] if len(Cm) == 1 else mx[Cm]

            # W layout: free = q*128 + m*8 + gl (K-order (m,gl)); sub writes
            # are 8-element step-1 runs spanning all 64 q (FD=512, 2x mode),
            # transpose inputs are contiguous 128-column blocks.
            wt = wpool.tile([128, NQ * 128], F16, tag="w")
            wv4 = wt.rearrange("p (q m gl) -> p q m gl", m=16, gl=8)
            wvr = wt.rearrange("p (q r) -> p q r", r=128)
            # m=0 columns: matmul contribution is zeroed by the zero params
            # rows but must be finite
            nc.gpsimd.memset(wv4[:, :, 0, :], 0.0)
            # NOTE: concurrent GpSimd tensor ops contend for SBUF ports and
            # drop DVE throughput ~3x (measured); keep all tensor work on DVE
            for m in range(1, 15):
                S = tuple(i for i in range(A) if (m >> i) & 1)
                Cm = tuple(i for i in range(A) if not ((m >> i) & 1))
                nc.vector.tensor_tensor(
                    wv4[:, :, m, :], sub_ap(S), sup_ap(Cm), mybir.AluOpType.subtract
                )
            # m15 = min of all four, written directly (no relu applied to it)
            nc.vector.tensor_tensor(
                wv4[:, :, 15, :], mn[(0, 1, 2)], s[3], mybir.AluOpType.min
            )
            # relu m=1..14 per q-half ([32,112] measured faster than [64,112])
            nc.vector.tensor_scalar_max(wvr[:, 0:32, 8:120], wvr[:, 0:32, 8:120], 0.0)
            nc.vector.tensor_scalar_max(wvr[:, 32:64, 8:120], wvr[:, 32:64, 8:120], 0.0)

            for qg in range(4):          # 4 groups of 16 channel-groups
                pt = ptp.tile([128, 16 * 128], F16, tag="pt")
                for j in range(16):
                    q = qg * 16 + j
                    nc.tensor.transpose(
                        pt[:, j * 128:(j + 1) * 128],
                        wt[:, q * 128:(q + 1) * 128],
                        identity[:],
                    )
                lt = lhsp.tile([128, 16 * 128], F16, tag="lt")
                # W^T evac split: 3 of 4 on DVE, 1 on ACT
                if qg == 3:
                    nc.scalar.copy(lt[:], pt[:])
                else:
                    nc.vector.tensor_copy(lt[:], pt[:])

                stg = stgp.tile([128, 16 * 256], F16, tag="stg")
                for seg in range(4):
                    pm = pmp.tile([128, 1024], F32, tag="pm")
                    for j2 in range(4):
                        j = seg * 4 + j2
                        q = qg * 16 + j
                        nc.tensor.matmul(
                            pm[:, j2 * 256:(j2 + 1) * 256],
                            lt[:, j * 128:(j + 1) * 128],
                            pbd[:, q * 256:(q + 1) * 256],
                            start=True,
                            stop=True,
                        )
                    dst = stg[:, seg * 1024:(seg + 1) * 1024]
                    nc.scalar.copy(dst, pm[:])
                    oev[0] += 1
                q0 = qg * 16
                nc.sync.dma_start(
                    out_d[bt * 128:(bt + 1) * 128, q0 * 256:(q0 + 16) * 256],
                    stg[:],
                )

    nc.compile()
    return nc


def _get_program():
    global _cached_nc
    if _cached_nc is None:
        _cached_nc = _build_program()
    return _cached_nc


def _make_inputs(X, params):
    X = np.ascontiguousarray(X, dtype=np.float32)
    params = np.ascontiguousarray(params, dtype=np.float32)
    P4 = params.reshape(NQ, 8, 16, O)                 # [q, gl, m, o]
    # block-diag table: pbd[m*8+gl, q*256 + gl*32 + o] = params[8q+gl, m, o]
    Pb = np.zeros((16, 8, NQ, 8, O), np.float32)
    for gl in range(8):
        Pb[1:, gl, :, gl, :] = P4[:, gl, 1:, :].transpose(1, 0, 2)
    pbd = np.ascontiguousarray(Pb.reshape(128, NQ * 256).astype(np.float16))
    # de-interleave X to per-arity fp16 planes: [B, G, A] -> [B, A, G]
    Xp = np.ascontiguousarray(
        X.reshape(B, G, A).transpose(0, 2, 1).astype(np.float16).reshape(B, A * G)
    )
    in_maps = [
        {"x": Xp[c * BS:(c + 1) * BS], "pbd": pbd}
        for c in range(NCORES)
    ]
    return in_maps


def kernel(X, params):
    nc = _get_program()
    in_maps = _make_inputs(X, params)
    res = run_bass_kernel_spmd(nc, in_maps, list(range(NCORES))).results
    out = np.concatenate(
        [res[c]["out"].astype(np.float32).reshape(BS, G, O) for c in range(NCORES)],
        axis=0,
    )
    return out


def kernel_traced(X, params):
    """Like kernel() but also returns the BassKernelResults (profile info)."""
    nc = _get_program()
    in_maps = _make_inputs(X, params)
    br = run_bass_kernel_spmd(nc, in_maps, list(range(NCORES)), trace=True)
    out = np.concatenate(
        [br.results[c]["out"].astype(np.float32).reshape(BS, G, O)
         for c in range(NCORES)],
        axis=0,
    )
    return out, br


# revision 14
# speedup vs baseline: 1.2475x; 1.0472x over previous
"""Trainium2 Bass kernel for nn_BatchHighOrderActivation.

Reference semantics (per batch b, channel g):
    sort the ARITY=4 values x = X[b,g,:], build barycentric coefficients from
    the sorted gaps, gather params rows by reverse-cumsum bitmasks, contract.

Sort/gather-free reformulation (multilinear simplex / Lovasz form):
    out[b,g,:] = sum_{m=1..15} w[b,g,m] * params[g,m,:]
    w[m] = relu( min_{i in m} x_i - max_{i not in m} x_i )   for m != 15
    w[15] = min_i x_i                                        (no relu)

Kernel structure per core (pure batch data-parallel sharding, 512 rows/core),
fp16 internal compute and fp16 DRAM output (host casts back to fp32; total
error ~7e-4 absmax-relative, well under the 2e-3 gate):
  - host: X de-interleaved to fp16 arity-planes grouped in b-tile PAIRS
          ([pair, p, a, t, g]); params expanded to an fp16 block-diagonal
          table (8 channels/group, K-order (m,gl))
  - DVE : subset min/max tree computed once per PAIR of b-tiles (FD=1024
          halves the per-op overhead share); 14 subtractions per b-tile as
          single FD=512 strided ops; quad-min written straight into the m15
          slots; relu per q-half. The first pair is processed in g-halves so
          the PE/ACT pipeline starts ~10us earlier.
  - PE  : transpose W 128x128 chunks via fp16 identity matmul (16 chunks per
          PSUM tile), then block-diag fp16 matmul (K=(m,gl)=128), fp32 PSUM
  - DVE : W^T evacuation (fp16 2x) for 12 of 16 tiles; ACT takes b-tile 0's
          (DVE is tree-busy early, ACT idle)
  - ACT : out evacuation PSUM->SBUF with fused fp32->fp16 cast; DVE takes a
          few of the last b-tile's (tail)
  - DMA : input loads ride SWDGE (gpsimd) queues; fp16 stores on HWDGE (sync)
NOTE: GpSimd tensor ops contend for SBUF ports and drop concurrent DVE
throughput ~3x (measured); keep GpSimd to memset + DMA descriptor work only.
Baseline from prior session: 122.5us (fp32 out, per-half subs, no pairing).
"""

import numpy as np
from contextlib import ExitStack

import concourse.bass as bass
import concourse.mybir as mybir
import concourse.tile as tile
from concourse import bacc
from concourse.bass_utils import run_bass_kernel_spmd
from concourse.masks import make_identity

F32 = mybir.dt.float32
F16 = mybir.dt.float16
NCORES = 8
B, G, A, O = 4096, 512, 4, 32
BS = B // NCORES        # 512 batch rows per core
NBT = BS // 128         # 4 b-tiles per core
NQ = G // 8             # 64 channel groups of 8

_PAIRS = [(0, 1), (0, 2), (0, 3), (1, 2), (1, 3), (2, 3)]
_TRIPLES = [(0, 1, 2), (0, 1, 3), (0, 2, 3), (1, 2, 3)]

_cached_nc = None


def _build_program():
    nc = bacc.Bacc("TRN2", target_bir_lowering=False, debug=False, num_devices=NCORES)

    # x: [pair*128 + p, (a, t, g)] fp16 — b-tile pairs interleaved on host
    x_d = nc.dram_tensor("x", [BS // 2, A * 2 * G], F16, kind="ExternalInput").ap()
    pbd_d = nc.dram_tensor("pbd", [128, NQ * 256], F16, kind="ExternalInput").ap()
    out_d = nc.dram_tensor("out", [BS, G * O], F16, kind="ExternalOutput").ap()

    with ExitStack() as ctx:
        tc = ctx.enter_context(tile.TileContext(nc))
        persist = ctx.enter_context(tc.tile_pool(name="persist", bufs=1))
        plpool = ctx.enter_context(tc.tile_pool(name="pl", bufs=2))
        treep = ctx.enter_context(tc.tile_pool(name="tree", bufs=1))
        wpool = ctx.enter_context(tc.tile_pool(name="w", bufs=2))
        lhsp = ctx.enter_context(tc.tile_pool(name="lt", bufs=4))
        stgp = ctx.enter_context(tc.tile_pool(name="stg", bufs=3))
        ptp = ctx.enter_context(tc.tile_pool(name="pt", bufs=1, space="PSUM"))
        pmp = ctx.enter_context(tc.tile_pool(name="pm", bufs=3, space="PSUM"))

        pbd = persist.tile([128, NQ * 256], F16)
        identity = persist.tile([128, 128], F16)
        pl0 = plpool.tile([128, A, 2, G], F16, tag="pl")
        nc.gpsimd.dma_start(
            pl0[:].rearrange("p a t g -> p (a t g)"), x_d[0:128, :]
        )
        nc.gpsimd.dma_start(pbd[:], pbd_d[:])
        make_identity(nc, identity[:])

        for pr in range(2):
            if pr == 0:
                pl = pl0
                halves = [(0, G // 2), (G // 2, G)]   # early pipeline start
            else:
                pl = plpool.tile([128, A, 2, G], F16, tag="pl")
                nc.gpsimd.dma_start(
                    pl[:].rearrange("p a t g -> p (a t g)"),
                    x_d[128:256, :],
                )
                halves = [(0, G)]

            tr = treep.tile([128, 20, 2, G], F16, tag="tree")
            wts = [
                wpool.tile([128, NQ * 128], F16, tag="w", name=f"wt_{pr}_{t}")
                for t in range(2)
            ]

            for (ga, gb) in halves:
                qa, qb = ga // 8, gb // 8
                s2 = [pl[:, i, :, ga:gb] for i in range(A)]      # [p, 2, gh]
                slot = [0]
                mn, mx = {}, {}

                def alloc():
                    ap = tr[:, slot[0], :, ga:gb]
                    slot[0] += 1
                    return ap

                # subset min/max tree, both b-tiles of the pair at once
                for (i, j) in _PAIRS:
                    mn[(i, j)] = alloc()
                    nc.vector.tensor_tensor(mn[(i, j)], s2[i], s2[j], mybir.AluOpType.min)
                for (i, j) in _PAIRS:
                    mx[(i, j)] = alloc()
                    nc.vector.tensor_tensor(mx[(i, j)], s2[i], s2[j], mybir.AluOpType.max)
                for (i, j, k) in _TRIPLES:
                    mn[(i, j, k)] = alloc()
                    nc.vector.tensor_tensor(mn[(i, j, k)], mn[(i, j)], s2[k], mybir.AluOpType.min)
                for (i, j, k) in _TRIPLES:
                    mx[(i, j, k)] = alloc()
                    nc.vector.tensor_tensor(mx[(i, j, k)], mx[(i, j)], s2[k], mybir.AluOpType.max)

                def sub_ap(S, t):
                    return s2[S[0]][:, t, :] if len(S) == 1 else mn[S][:, t, :]

                def sup_ap(Cm, t):
                    return s2[Cm[0]][:, t, :] if len(Cm) == 1 else mx[Cm][:, t, :]

                for t in range(2):
                    bt = pr * 2 + t
                    wt = wts[t]
                    wv4 = wt.rearrange("p (q m gl) -> p q m gl", m=16, gl=8)
                    wvr = wt.rearrange("p (q r) -> p q r", r=128)
                    nc.gpsimd.memset(wv4[:, qa:qb, 0, :], 0.0)
                    for m in range(1, 15):
                        S = tuple(i for i in range(A) if (m >> i) & 1)
                        Cm = tuple(i for i in range(A) if not ((m >> i) & 1))
                        nc.vector.tensor_tensor(
                            wv4[:, qa:qb, m, :], sub_ap(S, t), sup_ap(Cm, t),
                            mybir.AluOpType.subtract,
                        )
                    nc.vector.tensor_tensor(
                        wv4[:, qa:qb, 15, :], mn[(0, 1, 2)][:, t, :], s2[3][:, t, :],
                        mybir.AluOpType.min,
                    )
                    # relu m=1..14 per 32-q chunk (measured faster than 64-q)
                    for q0 in range(qa, qb, 32):
                        nc.vector.tensor_scalar_max(
                            wvr[:, q0:q0 + 32, 8:120], wvr[:, q0:q0 + 32, 8:120], 0.0
                        )

                    for qg in range(qa // 16, qb // 16):
                        pt = ptp.tile([128, 16 * 128], F16, tag="pt")
                        for j in range(16):
                            q = qg * 16 + j
                            nc.tensor.transpose(
                                pt[:, j * 128:(j + 1) * 128],
                                wt[:, q * 128:(q + 1) * 128],
                                identity[:],
                            )
                        lt = lhsp.tile([128, 16 * 128], F16, tag="lt")
                        # W^T evac: b-tile 0 on ACT (DVE tree-busy, ACT idle),
                        # later tiles on DVE
                        if bt == 0:
                            nc.scalar.copy(lt[:], pt[:])
                        else:
                            nc.vector.tensor_copy(lt[:], pt[:])

                        stg = stgp.tile([128, 16 * 256], F16, tag="stg")
                        for seg in range(4):
                            pm = pmp.tile([128, 1024], F32, tag="pm")
                            for j2 in range(4):
                                j = seg * 4 + j2
                                q = qg * 16 + j
                                nc.tensor.matmul(
                                    pm[:, j2 * 256:(j2 + 1) * 256],
                                    lt[:, j * 128:(j + 1) * 128],
                                    pbd[:, q * 256:(q + 1) * 256],
                                    start=True,
                                    stop=True,
                                )
                            dst = stg[:, seg * 1024:(seg + 1) * 1024]
                            # tail help: DVE takes half of the last qg's evacs
                            if bt == 3 and qg == 3 and seg >= 2:
                                nc.vector.tensor_copy(dst, pm[:])
                            else:
                                nc.scalar.copy(dst, pm[:])
                        q0 = qg * 16
                        nc.sync.dma_start(
                            out_d[bt * 128:(bt + 1) * 128, q0 * 256:(q0 + 16) * 256],
                            stg[:],
                        )

    nc.compile()
    return nc


def _get_program():
    global _cached_nc
    if _cached_nc is None:
        _cached_nc = _build_program()
    return _cached_nc


def _make_inputs(X, params):
    X = np.ascontiguousarray(X, dtype=np.float32)
    params = np.ascontiguousarray(params, dtype=np.float32)
    P4 = params.reshape(NQ, 8, 16, O)                 # [q, gl, m, o]
    # block-diag table: pbd[m*8+gl, q*256 + gl*32 + o] = params[8q+gl, m, o]
    Pb = np.zeros((16, 8, NQ, 8, O), np.float32)
    for gl in range(8):
        Pb[1:, gl, :, gl, :] = P4[:, gl, 1:, :].transpose(1, 0, 2)
    pbd = np.ascontiguousarray(Pb.reshape(128, NQ * 256).astype(np.float16))
    # X per core: [512b, G, A] -> [pair, p, a, t, g] fp16 planes
    Xp = (X.reshape(NCORES, 2, 2, 128, G, A)          # [c, pair, t, p, g, a]
            .transpose(0, 1, 3, 5, 2, 4)              # [c, pair, p, a, t, g]
            .astype(np.float16)
            .reshape(NCORES, 256, A * 2 * G))
    Xp = np.ascontiguousarray(Xp)
    in_maps = [
        {"x": Xp[c], "pbd": pbd}
        for c in range(NCORES)
    ]
    return in_maps


def kernel(X, params):
    nc = _get_program()
    in_maps = _make_inputs(X, params)
    res = run_bass_kernel_spmd(nc, in_maps, list(range(NCORES))).results
    out = np.concatenate(
        [res[c]["out"].astype(np.float32).reshape(BS, G, O) for c in range(NCORES)],
        axis=0,
    )
    return out


def kernel_traced(X, params):
    """Like kernel() but also returns the BassKernelResults (profile info)."""
    nc = _get_program()
    in_maps = _make_inputs(X, params)
    br = run_bass_kernel_spmd(nc, in_maps, list(range(NCORES)), trace=True)
    out = np.concatenate(
        [br.results[c]["out"].astype(np.float32).reshape(BS, G, O)
         for c in range(NCORES)],
        axis=0,
    )
    return out, br
